# revision 7
# baseline (speedup 1.0000x reference)
"""Trainium2 Bass kernel for nn_BranchingLayer (gnn_message_passing).

Computation (reference):
    parents_ftxs = x[idxs_level]                      # identity gather (arange)
    pg           = global_features[parents_idxs % B]  # random gather
    h1 = leaky_relu([parents_ftxs, pg] @ W1 + b1)
    h2 = h1 @ W2 + b2 + repeat(parents_ftxs, 2, -1)
    children = interleave-reshape(h2)                 # [(2p+br)*B + b, f] = h2[p*B+b, br*F+f]
    out = concat([x, children])

Device strategy (8 cores, rows sharded 32768/core = 32 parents):
  - host: gather pg rows and pre-transpose to [64, rows] per core
  - per 512-row group: DMA x rows + pgT cols; PE-transpose x tiles; fp32r matmuls
    (mm1 -> h1^T in PSUM, leaky-relu on ACT -> SBUF, mm2 row-major with W2 streamed
     and h1^T/x^T as stationary; residual folded in as a third matmul against a
     0/1 repeat matrix); DVE adds b2 while splitting branch channels; DMA children out.
  - host: concat [x, children].
"""

import sys

import numpy as np

if "/opt/trn_rl_repo" not in sys.path:
    sys.path.insert(0, "/opt/trn_rl_repo")

N_PARENTS = 256
BATCH = 1024
N_FEAT = 128
N_BR = 2
N_GLOBAL = 64
N_CORES = 8
ROWS = N_PARENTS * BATCH            # 262144
RPC = ROWS // N_CORES               # 32768 rows per core
CPC = RPC * N_BR                    # 65536 child rows per core
GROUP = 512                         # rows per pipeline group
N_GROUPS = RPC // GROUP             # 64
HID = 256

# leaky-relu implementation: "lrelu" (single ACT op, HW Lrelu table),
# "relu2" (Identity + Relu + DVE add, guaranteed semantics)
LRELU_MODE = "relu2"

_CACHE = {}


def _split_multiwait(nc, mybir):
    """This image's walrus accepts only one sync-wait per instruction; hoist
    extra waits onto same-engine NOPs inserted before the instruction."""
    for f in nc.m.functions:
        for bb in f.blocks:
            new_insts = []
            changed = False
            for inst in bb.instructions:
                si = inst.sync_info
                if si is not None and len(si.on_wait) > 1:
                    waits = list(si.on_wait)
                    for w in waits[:-1]:
                        new_insts.append(
                            mybir.InstNoOp(
                                name=nc.get_next_instruction_name(),
                                engine=inst.engine,
                                sync_info=mybir.SyncInfo(on_wait=[w], on_update=[]),
                            )
                        )
                    inst.sync_info = mybir.SyncInfo(
                        on_wait=[waits[-1]], on_update=list(si.on_update)
                    )
                    changed = True
                new_insts.append(inst)
            if changed:
                bb.instructions = new_insts


def _build_program(lrelu_mode=LRELU_MODE, split_waits=True):
    key = ("prog", lrelu_mode, split_waits)
    if key in _CACHE:
        return _CACHE[key]

    import concourse.bass as bass
    import concourse.mybir as mybir
    import concourse.tile as tile

    f32 = mybir.dt.float32
    f32r = mybir.dt.float32r

    def r(ap):
        return ap.bitcast(f32r)

    nc = bass.Bass()
    xs = nc.declare_dram_parameter("xs", [RPC, N_FEAT], f32, isOutput=False)
    pgt = nc.declare_dram_parameter("pgt", [N_GLOBAL, RPC], f32r, isOutput=False)
    w1 = nc.declare_dram_parameter("w1", [N_FEAT + N_GLOBAL, HID], f32r, isOutput=False)
    b1c = nc.declare_dram_parameter("b1c", [128, 2], f32, isOutput=False)
    b1n = nc.declare_dram_parameter("b1n", [128, 2], f32, isOutput=False)
    w2 = nc.declare_dram_parameter("w2", [HID, HID], f32r, isOutput=False)
    b2t = nc.declare_dram_parameter("b2t", [128, HID], f32, isOutput=False)
    rmat = nc.declare_dram_parameter("rmat", [N_FEAT, HID], f32r, isOutput=False)
    ident = nc.declare_dram_parameter("ident", [128, 128], f32, isOutput=False)
    ch = nc.declare_dram_parameter("ch", [CPC, N_FEAT], f32, isOutput=True)

    AF = mybir.ActivationFunctionType

    with tile.TileContext(nc) as tc:
        with (
            tc.tile_pool(name="const", bufs=1) as cpool,
            tc.tile_pool(name="xin", bufs=3) as xpool,
            tc.tile_pool(name="pg", bufs=3) as gpool,
            tc.tile_pool(name="xt", bufs=2) as xtpool,
            tc.tile_pool(name="h1", bufs=2) as h1pool,
            tc.tile_pool(name="tmp", bufs=2) as tpool,
            tc.tile_pool(name="cout", bufs=3) as opool,
            tc.tile_pool(name="psA", bufs=2, space="PSUM") as psA,
            tc.tile_pool(name="psB", bufs=2, space="PSUM") as psB,
            tc.tile_pool(name="psC", bufs=4, space="PSUM") as psC,
        ):
            w1a = cpool.tile([128, HID], f32r)
            nc.sync.dma_start(w1a[:], w1[0:128, :])
            w1b = cpool.tile([64, HID], f32r)
            nc.sync.dma_start(w1b[:], w1[128:192, :])
            w2a = cpool.tile([128, HID], f32r)
            nc.sync.dma_start(w2a[:], w2[0:128, :])
            w2b = cpool.tile([128, HID], f32r)
            nc.sync.dma_start(w2b[:], w2[128:256, :])
            rm = cpool.tile([128, HID], f32r)
            nc.sync.dma_start(rm[:], rmat[:])
            b2s = cpool.tile([128, HID], f32)
            nc.sync.dma_start(b2s[:], b2t[:])
            b1s = cpool.tile([128, 2], f32)
            nc.sync.dma_start(b1s[:], b1c[:])
            b1ns = cpool.tile([128, 2], f32)
            nc.sync.dma_start(b1ns[:], b1n[:])
            idn = cpool.tile([128, 128], f32)
            nc.sync.dma_start(idn[:], ident[:])

            for g in range(N_GROUPS):
                # ---- load x rows (4 subtiles of 128 rows) and pgT columns ----
                xg = xpool.tile([128, GROUP], f32)
                nc.sync.dma_start(
                    xg[:, :].rearrange("p (s f) -> p s f", s=4),
                    xs[g * GROUP:(g + 1) * GROUP, :].rearrange("(s p) f -> p s f", p=128),
                )
                pgg = gpool.tile([64, GROUP], f32r)
                nc.sync.dma_start(pgg[:, :], pgt[:, g * GROUP:(g + 1) * GROUP])

                # ---- transpose x subtiles: [128 rows, 128 feat] -> [feat, rows] ----
                xt_ps = psA.tile([128, GROUP], f32)
                for s in range(4):
                    nc.tensor.transpose(
                        xt_ps[:, s * 128:(s + 1) * 128],
                        xg[:, s * 128:(s + 1) * 128],
                        idn[:, :],
                    )
                xt = xtpool.tile([128, GROUP], f32r)
                nc.scalar.copy(xt[:, :], xt_ps[:, :])

                # ---- mm1: h1^T[c, rows] = W1a^T @ x^T + W1b^T @ pg^T ----
                h1ps = [psB.tile([128, GROUP], f32, tag="h1ps", name=f"h1ps{m_}") for m_ in range(2)]
                for m in range(2):
                    msl = slice(m * 128, (m + 1) * 128)
                    nc.tensor.matmul(
                        h1ps[m][:, :], w1a[:, msl], xt[:, :],
                        start=True, stop=False,
                    )
                    nc.tensor.matmul(
                        h1ps[m][:, :], w1b[:, msl], pgg[:, :],
                        start=False, stop=True,
                    )

                # ---- leaky relu + bias -> SBUF ----
                h1 = [h1pool.tile([128, GROUP], f32r, tag="h1sb", name=f"h1sb{m_}") for m_ in range(2)]
                for m in range(2):
                    if lrelu_mode == "lrelu":
                        nc.scalar.activation(
                            h1[m][:, :], h1ps[m][:, :], AF.Lrelu,
                            bias=b1s[:, m:m + 1], scale=1.0, alpha=0.01,
                        )
                    else:
                        # leaky(u) = u + 0.99*relu(-u), u = x + b1
                        v = tpool.tile([128, GROUP], f32, tag="v")
                        nc.scalar.activation(
                            h1[m][:, :], h1ps[m][:, :], AF.Identity,
                            bias=b1s[:, m:m + 1],
                        )
                        nc.scalar.activation(
                            v[:, :], h1ps[m][:, :], AF.Relu,
                            bias=b1ns[:, m:m + 1], scale=-0.99,
                        )
                        nc.vector.tensor_add(h1[m][:, :], h1[m][:, :], v[:, :])

                # ---- mm2 per 128-row subtile (row-major out) + residual ----
                cht = opool.tile([128, 2 * GROUP], f32)
                cht3 = cht[:, :].rearrange("p (b k) -> p b k", b=2)
                b2s3 = b2s[:, :].rearrange("p (b k) -> p b k", b=2)
                for s in range(4):
                    ssl = slice(s * 128, (s + 1) * 128)
                    h2ps = psC.tile([128, HID], f32)
                    nc.tensor.matmul(
                        h2ps[:, :], h1[0][:, ssl], w2a[:, :],
                        start=True, stop=False,
                    )
                    nc.tensor.matmul(
                        h2ps[:, :], h1[1][:, ssl], w2b[:, :],
                        start=False, stop=False,
                    )
                    nc.tensor.matmul(
                        h2ps[:, :], xt[:, ssl], rm[:, :],
                        start=False, stop=True,
                    )
                    # children[:, br, k] = h2 + b2, split into branch halves of cht
                    nc.vector.tensor_add(
                        cht3[:, :, ssl],
                        h2ps[:, :].rearrange("p (b k) -> p b k", b=2),
                        b2s3[:, :, :],
                    )

                # ---- store children: branch-contiguous blocks of 512 rows ----
                p_local, half = divmod(g, 2)
                base0 = (2 * p_local) * BATCH + half * GROUP
                base1 = (2 * p_local + 1) * BATCH + half * GROUP
                nc.sync.dma_start(
                    ch[base0:base0 + GROUP, :].rearrange("(s p) f -> p s f", p=128),
                    cht[:, 0:GROUP].rearrange("p (s f) -> p s f", s=4),
                )
                nc.sync.dma_start(
                    ch[base1:base1 + GROUP, :].rearrange("(s p) f -> p s f", p=128),
                    cht[:, GROUP:2 * GROUP].rearrange("p (s f) -> p s f", s=4),
                )

    if split_waits:
        _split_multiwait(nc, mybir)
    _CACHE[key] = nc
    return nc


def _host_prep(x, global_features, W1, b1, W2, b2, idxs_level, parents_idxs):
    x = np.ascontiguousarray(np.asarray(x, dtype=np.float32))
    G = np.asarray(global_features, dtype=np.float32)
    W1 = np.ascontiguousarray(np.asarray(W1, dtype=np.float32))
    b1 = np.asarray(b1, dtype=np.float32)
    W2 = np.ascontiguousarray(np.asarray(W2, dtype=np.float32))
    b2 = np.asarray(b2, dtype=np.float32)
    idxs = np.asarray(idxs_level)
    pidx = np.asarray(parents_idxs)

    if np.array_equal(idxs, np.arange(ROWS, dtype=idxs.dtype)):
        xg = x
    else:  # general gather fallback (host)
        xg = np.ascontiguousarray(x[idxs])

    pg = G[pidx % BATCH]                              # [ROWS, 64]
    pgt = np.ascontiguousarray(
        pg.reshape(N_CORES, RPC, N_GLOBAL).transpose(0, 2, 1)
    )                                                 # [8, 64, RPC]

    b1c = np.ascontiguousarray(b1.reshape(2, 128).T)  # [128, 2]
    b1n = np.ascontiguousarray((-0.99 * b1).reshape(2, 128).T)
    b2t = np.ascontiguousarray(np.broadcast_to(b2, (128, HID)))
    rmat = np.zeros((N_FEAT, HID), dtype=np.float32)
    k = np.arange(N_FEAT)
    rmat[k, 2 * k] = 1.0
    rmat[k, 2 * k + 1] = 1.0
    ident = np.eye(128, dtype=np.float32)

    in_maps = []
    for c in range(N_CORES):
        in_maps.append({
            "xs": xg[c * RPC:(c + 1) * RPC],
            "pgt": pgt[c],
            "w1": W1,
            "b1c": b1c,
            "b1n": b1n,
            "w2": W2,
            "b2t": b2t,
            "rmat": rmat,
            "ident": ident,
        })
    return x, in_maps


def kernel(x, global_features, W1, b1, W2, b2, idxs_level, parents_idxs,
           _trace=False, _trace_kwargs=None):
    from concourse.bass_utils import run_bass_kernel_spmd

    x_np, in_maps = _host_prep(
        x, global_features, W1, b1, W2, b2, idxs_level, parents_idxs
    )
    nc = _build_program()
    res = run_bass_kernel_spmd(
        nc, in_maps, list(range(N_CORES)),
        trace=_trace, **(_trace_kwargs or {}),
    )
    children = np.concatenate(
        [res.results[c]["ch"] for c in range(N_CORES)], axis=0
    )
    out = np.concatenate([x_np, children], axis=0)
    if _trace:
        kernel.last_result = res
    return out
